# revision 34
# baseline (speedup 1.0000x reference)
"""Trainium2 Bass kernel: RoPE causal attention (B=1,S=2048,D=4096,H=32).

Tensor-parallel over heads on 8 NeuronCores: core c owns heads [4c,4c+4).
Fully SBUF-resident single-pass design:
  - host pre-casts to bf16 and pre-transposes (xT, wqT/wkT/wvT, woT), and
    pre-permutes wq/wk rows so RoPE pairs become partition halves 0-63/64-127;
    cos/sin computed on host (partition-duplicated).
  - projections qT/kT/vS computed from xT chunks with RoPE fused into the
    PSUM->SBUF epilogue (4 DVE ops per tile).
  - attention computes scores^T [sk,sq] directly so exp output feeds the PV
    matmul as the stationary operand with no probs transpose; a ones column
    appended to v yields softmax row-sums for free; normalization is a
    per-partition scalar multiply.
  - y = att @ wo_c.T accumulated per 128x512 tile in PSUM, bf16 out.
Host sums the 8 partial outputs.
"""

import math
import numpy as np
import ml_dtypes

import concourse.bass as bass
import concourse.mybir as mybir
import concourse.tile as tile
from concourse import bacc
from concourse.bass import ts, ds
from concourse.bass_utils import run_bass_kernel_spmd
from concourse.masks import make_identity

B, S, D, H, HD = 1, 2048, 4096, 32, 128
NCORES = 8
HL = H // NCORES          # 4 heads per core
DL = HL * HD              # 512 local head dims
NT = S // 128             # 16 seq tiles
NK = D // 128             # 32 contraction tiles
SCALE = 1.0 / math.sqrt(HD)
F32 = mybir.dt.float32
BF16 = mybir.dt.bfloat16
MUL = mybir.AluOpType.mult
SUB = mybir.AluOpType.subtract
ADD = mybir.AluOpType.add

_CACHE = {}


def _build():
    nc = bacc.Bacc(None, target_bir_lowering=False, debug=False)
    xT_t = nc.dram_tensor("xT", [D, S], BF16, kind="ExternalInput")
    wqT_t = nc.dram_tensor("wqT", [D, DL], BF16, kind="ExternalInput")
    wkT_t = nc.dram_tensor("wkT", [D, DL], BF16, kind="ExternalInput")
    wvT_t = nc.dram_tensor("wvT", [D, DL], BF16, kind="ExternalInput")
    woT_t = nc.dram_tensor("woT", [DL, D], BF16, kind="ExternalInput")
    cos_t = nc.dram_tensor("cosF", [128, S], BF16, kind="ExternalInput")
    sin_t = nc.dram_tensor("sinF", [128, S], BF16, kind="ExternalInput")
    ltm_t = nc.dram_tensor("ltm", [128, 128], BF16, kind="ExternalInput")
    y_t = nc.dram_tensor("y", [S, D], BF16, kind="ExternalOutput")

    EXP = mybir.ActivationFunctionType.Exp
    CPY = mybir.ActivationFunctionType.Copy

    with tile.TileContext(nc) as tc:
        with tc.tile_pool(name="pers", bufs=1) as pers:
            qT = pers.tile([128, HL, S], BF16)    # [hd, h, s], hd pair-permuted
            kT = pers.tile([128, HL, S], BF16)
            vS = pers.tile([128, NT, HL, 130], BF16)  # [sk%128, sk//128, h, hd+ones]
            cosF = pers.tile([128, NT, 128], BF16)  # [s%128, s//128, pair dup x2]
            sinF = pers.tile([128, NT, 128], BF16)
            ident = pers.tile([128, 128], BF16)
            ltm = pers.tile([128, 128], BF16)

            make_identity(nc, ident)
            nc.vector.memset(vS[:, :, :, 128:129], 1.0)

            # ---- stage 1: projections with fused RoPE ----
            # q/k/v all computed per sq-tile as [sq, dl] (lhsT = xT slice,
            # rhs = full weight). RoPE is applied along the free dim (pairs
            # deinterleaved per head by the host weight permutation), then
            # q/k tiles are PE-transposed into qT/kT [hd, s].
            with (
                tc.tile_pool(name="wts", bufs=1) as wp,
                tc.tile_pool(name="xs", bufs=2) as xsp,
                tc.tile_pool(name="rsc", bufs=3) as rp,
                tc.tile_pool(name="pp", bufs=3, space="PSUM") as pp,
                tc.tile_pool(name="tq", bufs=2, space="PSUM") as tqp,
            ):
                wq_sb = wp.tile([128, NK, DL], BF16)
                wk_sb = wp.tile([128, NK, DL], BF16)
                wv_sb = wp.tile([128, NK, DL], BF16)
                # Batched loads, chunked so the first matmuls start after only
                # a few hundred KB while the rest streams in.
                xs0 = xsp.tile([128, NK, 256], BF16, tag="xs")

                def load_chunked(dst, src_ap, nk, pieces):
                    per = nk // pieces
                    for i in range(pieces):
                        nc.sync.dma_start(
                            out=dst[:, ts(i, per)],
                            in_=src_ap[ds(i * per * 128, per * 128), :].rearrange(
                                "(k p) c -> p k c", p=128))

                for i in range(8):  # interleave xs0/wq pieces: first mm ~2us in
                    nc.sync.dma_start(
                        out=xs0[:, ts(i, 4)],
                        in_=xT_t.ap()[ds(i * 512, 512), 0:256].rearrange(
                            "(k p) c -> p k c", p=128))
                    nc.sync.dma_start(
                        out=wq_sb[:, ts(i, 4)],
                        in_=wqT_t.ap()[ds(i * 512, 512), :].rearrange(
                            "(k p) c -> p k c", p=128))
                    if i == 1:
                        # rope constants are first needed ~10us in
                        nc.sync.dma_start(out=cosF[:], in_=cos_t[:, :])
                        nc.sync.dma_start(out=sinF[:], in_=sin_t[:, :])
                        nc.sync.dma_start(out=ltm[:], in_=ltm_t[:, :])
                load_chunked(wk_sb, wkT_t.ap(), NK, 4)
                load_chunked(wv_sb, wvT_t.ap(), NK, 4)

                pending = []

                def drain_pending():
                    while pending:
                        src, dstT, h, st = pending.pop(0)
                        tpq = tqp.tile([128, 128], BF16, tag="tpq")
                        nc.tensor.transpose(tpq[:], src[:, ts(h, 128)], ident[:])
                        nc.vector.tensor_copy(out=dstT[:, h, ts(st, 128)],
                                              in_=tpq[:])

                for c2 in range(8):
                    if c2 == 0:
                        xs = xs0
                    else:
                        xs = xsp.tile([128, NK, 256], BF16, tag="xs")
                        nc.sync.dma_start(
                            out=xs[:],
                            in_=xT_t.ap()[:, ds(c2 * 256, 256)].rearrange(
                                "(k p) c -> p k c", p=128))
                    for stl in range(2):
                        st = c2 * 2 + stl
                        for pi, w_sb in ((0, wq_sb), (1, wk_sb), (2, wv_sb)):
                            ps = pp.tile([128, 512], F32, tag="ps")
                            for k in range(NK):
                                nc.tensor.matmul(ps[:], xs[:, k, ts(stl, 128)],
                                                 w_sb[:, k], start=(k == 0),
                                                 stop=(k == NK - 1))
                            drain_pending()
                            if pi < 2:
                                dstT = qT if pi == 0 else kT
                                qk_sb = rp.tile([128, DL], BF16, tag="qksb")
                                for h in range(HL):
                                    p1 = rp.tile([128, 128], F32, tag="p1")
                                    p2 = rp.tile([128, 128], F32, tag="p2")
                                    nc.vector.tensor_tensor(out=p1[:], in0=ps[:, ts(h, 128)], in1=cosF[:, st], op=MUL)
                                    nc.vector.tensor_tensor(out=p2[:], in0=ps[:, ts(h, 128)], in1=sinF[:, st], op=MUL)
                                    nc.vector.tensor_tensor(out=qk_sb[:, ds(h * 128, 64)], in0=p1[:, 0:64], in1=p2[:, 64:128], op=SUB)
                                    nc.vector.tensor_tensor(out=qk_sb[:, ds(h * 128 + 64, 64)], in0=p2[:, 0:64], in1=p1[:, 64:128], op=ADD)
                                for h in range(HL):
                                    pending.append((qk_sb, dstT, h, st))
                            else:
                                for h in range(HL):
                                    nc.vector.tensor_copy(out=vS[:, st, h, 0:128],
                                                          in_=ps[:, ts(h, 128)])
                drain_pending()

            # ---- stage 2: causal attention + wo ----
            with (
                tc.tile_pool(name="att2", bufs=1) as ap2,
                tc.tile_pool(name="esc", bufs=4) as esc,
                tc.tile_pool(name="stt", bufs=4) as stp,
                tc.tile_pool(name="ysb", bufs=4) as ysb,
                tc.tile_pool(name="scp", bufs=3, space="PSUM") as scp,
                tc.tile_pool(name="attp", bufs=2, space="PSUM") as attp,
                tc.tile_pool(name="tpp", bufs=1, space="PSUM") as tpp,
                tc.tile_pool(name="wop", bufs=2, space="PSUM") as wop,
            ):
                woT_sb = ap2.tile([128, HL, D], BF16)
                attT = ap2.tile([128, HL, S], BF16)
                nc.sync.dma_start(
                    out=woT_sb[:], in_=woT_t.ap().rearrange("(j p) c -> p j c", p=128))

                # wo matmul groups from the previous chunk, interleaved into
                # the next chunk's s-loops so PE fills exp-wait gaps.
                wo_pending = []
                # deferred normalize/transpose epilogues: drained after the
                # NEXT head's first score pair so their DVE latency hides
                # under PE score work.
                norm_pending = []

                def drain_norms():
                    while norm_pending:
                        aps, hh, tq = norm_pending.pop(0)
                        rinv = stp.tile([128, 1], F32, tag="rinv")
                        nc.vector.reciprocal(rinv[:], aps[:, 128:129])
                        a_sb = esc.tile([128, 128], BF16, tag="asb")
                        nc.vector.tensor_scalar_mul(a_sb[:], aps[:, 0:128], rinv[:])
                        tp = tpp.tile([128, 128], BF16, tag="tp")
                        nc.tensor.transpose(tp[:], a_sb[:], ident[:])
                        nc.vector.tensor_copy(out=attT[:, hh, ts(tq, 128)], in_=tp[:])

                # Each wo group is emitted as two 2-matmul halves so there is
                # enough gap-filler for every exp-wait slot (224 halves vs 140
                # slots; whole groups only number 112).
                wo_cur = []

                def emit_wo(n):
                    while n and (wo_pending or wo_cur):
                        n -= 1
                        if not wo_cur:
                            tq, dc = wo_pending.pop(0)
                            wops = wop.tile([128, 512], F32, tag="wo")
                            for j in (0, 1):
                                nc.tensor.matmul(wops[:], attT[:, j, ts(tq, 128)],
                                                 woT_sb[:, j, ds(dc * 512, 512)],
                                                 start=(j == 0), stop=False)
                            wo_cur.append((tq, dc, wops))
                        else:
                            tq, dc, wops = wo_cur.pop()
                            for j in (2, 3):
                                nc.tensor.matmul(wops[:], attT[:, j, ts(tq, 128)],
                                                 woT_sb[:, j, ds(dc * 512, 512)],
                                                 start=False, stop=(j == HL - 1))
                            y_sb = ysb.tile([128, 512], BF16, tag="y")
                            if dc % 2 == 0:
                                nc.scalar.activation(y_sb[:], wops[:], CPY)
                            else:
                                nc.vector.tensor_copy(out=y_sb[:], in_=wops[:])
                            nc.sync.dma_start(
                                out=y_t[ts(tq, 128), ds(dc * 512, 512)], in_=y_sb[:])

                for c8 in range(8):
                    base_tq = 2 * c8
                    for h in range(HL):
                        nps = c8 + 1  # sk-tile pairs
                        att_ps = [attp.tile([128, 129], F32, tag="attps",
                                            name=f"attps{i}")
                                  for i in range(2)]

                        def emit_sc(sp):
                            # scores^T + exp for sk-tile pair (2sp, 2sp+1)
                            sc = scp.tile([128, 512], F32, tag="sc")
                            for b in range(2):
                                nc.tensor.matmul(sc[:, ts(b, 256)],
                                                 kT[:, h, ts(2 * sp + b, 128)],
                                                 qT[:, h, ds(c8 * 256, 256)],
                                                 start=True, stop=True)
                            e = esc.tile([128, 512], BF16, tag="e")
                            nc.scalar.activation(e[:], sc[:], EXP, scale=SCALE)
                            if sp == c8:  # diagonal blocks
                                nc.vector.tensor_tensor(out=e[:, 0:128], in0=e[:, 0:128], in1=ltm[:], op=MUL)
                                nc.vector.tensor_tensor(out=e[:, 384:512], in0=e[:, 384:512], in1=ltm[:], op=MUL)
                            return e

                        def emit_pv(sp, e):
                            for b in range(2):
                                s = 2 * sp + b
                                for ti in range(2):
                                    tq = base_tq + ti
                                    if tq >= s:
                                        nc.tensor.matmul(
                                            att_ps[ti][:],
                                            e[:, ds(b * 256 + ti * 128, 128)],
                                            vS[:, s, h, 0:129],
                                            start=(s == 0), stop=(s == tq))

                        # 2-pair-deep score/exp pipeline: PE stays ~2 exp
                        # latencies ahead of the PV consumer.
                        es = [emit_sc(sp) for sp in range(min(2, nps))]
                        drain_norms()
                        for sp in range(nps):
                            if sp + 2 < nps:
                                es.append(emit_sc(sp + 2))
                            emit_pv(sp, es[sp])
                            emit_wo(1)

                        for ti in range(2):
                            norm_pending.append((att_ps[ti], h, base_tq + ti))

                    for ti in range(2):
                        for dc in range(8):
                            wo_pending.append((base_tq + ti, dc))
                drain_norms()
                emit_wo(2 * len(wo_pending) + len(wo_cur))

    nc.compile()
    return nc


def _rope_perm():
    """Row permutation putting even rope components in rows 0-63, odd in 64-127
    of each head block."""
    perm = np.empty(DL, dtype=np.int64)
    for r in range(DL):
        h, p = divmod(r, 128)
        perm[r] = h * 128 + (2 * p if p < 64 else 2 * (p - 64) + 1)
    return perm


def _prep_inputs(x, freqs, wq, wk, wv, wo):
    bf16 = ml_dtypes.bfloat16
    x2 = np.asarray(x, dtype=np.float32).reshape(S, D)
    xT = np.ascontiguousarray(x2.T).astype(bf16)
    f64 = np.asarray(freqs, dtype=np.float64)
    # cosF[p, st*128 + j] = cos(freqs[st*128 + p, j % 64]); pairs duplicated
    # along the free dim so one [128,128] tile covers a whole head.
    cosP = np.cos(f64).reshape(NT, 128, 64)
    sinP = np.sin(f64).reshape(NT, 128, 64)
    cos2 = np.ascontiguousarray(
        np.concatenate([cosP, cosP], axis=2).transpose(1, 0, 2).reshape(128, S)
    ).astype(bf16)
    sin2 = np.ascontiguousarray(
        np.concatenate([sinP, sinP], axis=2).transpose(1, 0, 2).reshape(128, S)
    ).astype(bf16)
    i = np.arange(128)
    ltm = (i[:, None] <= i[None, :]).astype(np.float32).astype(bf16)
    perm = _rope_perm()
    in_maps = []
    for c in range(NCORES):
        sl = slice(c * DL, (c + 1) * DL)
        wq_c = np.asarray(wq, np.float32)[sl, :][perm, :]
        wk_c = np.asarray(wk, np.float32)[sl, :][perm, :]
        wv_c = np.asarray(wv, np.float32)[sl, :]
        wo_c = np.asarray(wo, np.float32)[:, sl]
        in_maps.append({
            "xT": xT,
            "wqT": np.ascontiguousarray(wq_c.T).astype(bf16),
            "wkT": np.ascontiguousarray(wk_c.T).astype(bf16),
            "wvT": np.ascontiguousarray(wv_c.T).astype(bf16),
            "woT": np.ascontiguousarray(wo_c.T).astype(bf16),
            "cosF": cos2,
            "sinF": sin2,
            "ltm": ltm,
        })
    return in_maps


def _run(inputs, trace=False):
    if "nc" not in _CACHE:
        _CACHE["nc"] = _build()
    nc = _CACHE["nc"]
    in_maps = _prep_inputs(**inputs)
    res = run_bass_kernel_spmd(nc, in_maps, core_ids=list(range(NCORES)), trace=trace)
    y = np.zeros((S, D), dtype=np.float64)
    for c in range(NCORES):
        y += res.results[c]["y"].astype(np.float64)
    return y.astype(np.float32).reshape(B, S, D), res.exec_time_ns


def kernel(**inputs):
    y, _ = _run(inputs, trace=False)
    return y


# revision 36
# speedup vs baseline: 1.0001x; 1.0001x over previous
"""Trainium2 Bass kernel: RoPE causal attention (B=1,S=2048,D=4096,H=32).

Tensor-parallel over heads on 8 NeuronCores: core c owns heads [4c,4c+4).
Fully SBUF-resident single-pass design:
  - host pre-casts to bf16 and pre-transposes (xT, wqT/wkT/wvT, woT), and
    pre-permutes wq/wk rows so RoPE pairs become partition halves 0-63/64-127;
    cos/sin computed on host (partition-duplicated).
  - projections qT/kT/vS computed from xT chunks with RoPE fused into the
    PSUM->SBUF epilogue (4 DVE ops per tile).
  - attention computes scores^T [sk,sq] directly so exp output feeds the PV
    matmul as the stationary operand with no probs transpose; a ones column
    appended to v yields softmax row-sums for free; normalization is a
    per-partition scalar multiply.
  - y = att @ wo_c.T accumulated per 128x512 tile in PSUM, bf16 out.
Host sums the 8 partial outputs.
"""

import math
import numpy as np
import ml_dtypes

import concourse.bass as bass
import concourse.mybir as mybir
import concourse.tile as tile
from concourse import bacc
from concourse.bass import ts, ds
from concourse.bass_utils import run_bass_kernel_spmd
from concourse.masks import make_identity

B, S, D, H, HD = 1, 2048, 4096, 32, 128
NCORES = 8
HL = H // NCORES          # 4 heads per core
DL = HL * HD              # 512 local head dims
NT = S // 128             # 16 seq tiles
NK = D // 128             # 32 contraction tiles
SCALE = 1.0 / math.sqrt(HD)
F32 = mybir.dt.float32
BF16 = mybir.dt.bfloat16
MUL = mybir.AluOpType.mult
SUB = mybir.AluOpType.subtract
ADD = mybir.AluOpType.add

_CACHE = {}


def _build():
    nc = bacc.Bacc(None, target_bir_lowering=False, debug=False)
    xT_t = nc.dram_tensor("xT", [D, S], BF16, kind="ExternalInput")
    wqT_t = nc.dram_tensor("wqT", [D, DL], BF16, kind="ExternalInput")
    wkT_t = nc.dram_tensor("wkT", [D, DL], BF16, kind="ExternalInput")
    wvT_t = nc.dram_tensor("wvT", [D, DL], BF16, kind="ExternalInput")
    woT_t = nc.dram_tensor("woT", [DL, D], BF16, kind="ExternalInput")
    cos_t = nc.dram_tensor("cosF", [128, S], BF16, kind="ExternalInput")
    sin_t = nc.dram_tensor("sinF", [128, S], BF16, kind="ExternalInput")
    ltm_t = nc.dram_tensor("ltm", [128, 128], BF16, kind="ExternalInput")
    y_t = nc.dram_tensor("y", [S, D], BF16, kind="ExternalOutput")

    EXP = mybir.ActivationFunctionType.Exp
    CPY = mybir.ActivationFunctionType.Copy

    with tile.TileContext(nc) as tc:
        with tc.tile_pool(name="pers", bufs=1) as pers:
            qT = pers.tile([128, HL, S], BF16)    # [hd, h, s], hd pair-permuted
            kT = pers.tile([128, HL, S], BF16)
            vS = pers.tile([128, NT, HL, 130], BF16)  # [sk%128, sk//128, h, hd+ones]
            cosF = pers.tile([128, NT, 128], BF16)  # [s%128, s//128, pair dup x2]
            sinF = pers.tile([128, NT, 128], BF16)
            ident = pers.tile([128, 128], BF16)
            ltm = pers.tile([128, 128], BF16)

            make_identity(nc, ident)
            nc.vector.memset(vS[:, :, :, 128:129], 1.0)

            # ---- stage 1: projections with fused RoPE ----
            # q/k/v all computed per sq-tile as [sq, dl] (lhsT = xT slice,
            # rhs = full weight). RoPE is applied along the free dim (pairs
            # deinterleaved per head by the host weight permutation), then
            # q/k tiles are PE-transposed into qT/kT [hd, s].
            with (
                tc.tile_pool(name="wts", bufs=1) as wp,
                tc.tile_pool(name="xs", bufs=2) as xsp,
                tc.tile_pool(name="rsc", bufs=3) as rp,
                tc.tile_pool(name="pp", bufs=3, space="PSUM") as pp,
                tc.tile_pool(name="tq", bufs=2, space="PSUM") as tqp,
            ):
                wq_sb = wp.tile([128, NK, DL], BF16)
                wk_sb = wp.tile([128, NK, DL], BF16)
                wv_sb = wp.tile([128, NK, DL], BF16)
                # Batched loads, chunked so the first matmuls start after only
                # a few hundred KB while the rest streams in.
                xs0 = xsp.tile([128, NK, 256], BF16, tag="xs")

                def load_chunked(dst, src_ap, nk, pieces):
                    per = nk // pieces
                    for i in range(pieces):
                        nc.sync.dma_start(
                            out=dst[:, ts(i, per)],
                            in_=src_ap[ds(i * per * 128, per * 128), :].rearrange(
                                "(k p) c -> p k c", p=128))

                for i in range(8):  # interleave xs0/wq pieces: first mm ~2us in
                    nc.sync.dma_start(
                        out=xs0[:, ts(i, 4)],
                        in_=xT_t.ap()[ds(i * 512, 512), 0:256].rearrange(
                            "(k p) c -> p k c", p=128))
                    nc.sync.dma_start(
                        out=wq_sb[:, ts(i, 4)],
                        in_=wqT_t.ap()[ds(i * 512, 512), :].rearrange(
                            "(k p) c -> p k c", p=128))
                    if i == 1:
                        # rope constants are first needed ~10us in
                        nc.sync.dma_start(out=cosF[:], in_=cos_t[:, :])
                        nc.sync.dma_start(out=sinF[:], in_=sin_t[:, :])
                        nc.sync.dma_start(out=ltm[:], in_=ltm_t[:, :])
                load_chunked(wk_sb, wkT_t.ap(), NK, 4)
                load_chunked(wv_sb, wvT_t.ap(), NK, 4)

                pending = []

                def drain_pending():
                    while pending:
                        src, dstT, h, st = pending.pop(0)
                        tpq = tqp.tile([128, 128], BF16, tag="tpq")
                        nc.tensor.transpose(tpq[:], src[:, ts(h, 128)], ident[:])
                        nc.vector.tensor_copy(out=dstT[:, h, ts(st, 128)],
                                              in_=tpq[:])

                for c2 in range(8):
                    if c2 == 0:
                        xs = xs0
                    else:
                        xs = xsp.tile([128, NK, 256], BF16, tag="xs")
                        nc.sync.dma_start(
                            out=xs[:],
                            in_=xT_t.ap()[:, ds(c2 * 256, 256)].rearrange(
                                "(k p) c -> p k c", p=128))
                    for stl in range(2):
                        st = c2 * 2 + stl
                        for pi, w_sb in ((0, wq_sb), (1, wk_sb), (2, wv_sb)):
                            ps = pp.tile([128, 512], F32, tag="ps")
                            for k in range(NK):
                                nc.tensor.matmul(ps[:], xs[:, k, ts(stl, 128)],
                                                 w_sb[:, k], start=(k == 0),
                                                 stop=(k == NK - 1))
                            drain_pending()
                            if pi < 2:
                                dstT = qT if pi == 0 else kT
                                qk_sb = rp.tile([128, DL], BF16, tag="qksb")
                                for h in range(HL):
                                    p1 = rp.tile([128, 128], F32, tag="p1")
                                    p2 = rp.tile([128, 128], F32, tag="p2")
                                    nc.vector.tensor_tensor(out=p1[:], in0=ps[:, ts(h, 128)], in1=cosF[:, st], op=MUL)
                                    nc.vector.tensor_tensor(out=p2[:], in0=ps[:, ts(h, 128)], in1=sinF[:, st], op=MUL)
                                    nc.vector.tensor_tensor(out=qk_sb[:, ds(h * 128, 64)], in0=p1[:, 0:64], in1=p2[:, 64:128], op=SUB)
                                    nc.vector.tensor_tensor(out=qk_sb[:, ds(h * 128 + 64, 64)], in0=p2[:, 0:64], in1=p1[:, 64:128], op=ADD)
                                for h in range(HL):
                                    pending.append((qk_sb, dstT, h, st))
                            else:
                                for h in range(HL):
                                    nc.vector.tensor_copy(out=vS[:, st, h, 0:128],
                                                          in_=ps[:, ts(h, 128)])
                drain_pending()

            # ---- stage 2: causal attention + wo ----
            with (
                tc.tile_pool(name="att2", bufs=1) as ap2,
                tc.tile_pool(name="esc", bufs=4) as esc,
                tc.tile_pool(name="stt", bufs=4) as stp,
                tc.tile_pool(name="ysb", bufs=4) as ysb,
                tc.tile_pool(name="scp", bufs=3, space="PSUM") as scp,
                tc.tile_pool(name="attp", bufs=2, space="PSUM") as attp,
                tc.tile_pool(name="tpp", bufs=1, space="PSUM") as tpp,
                tc.tile_pool(name="wop", bufs=2, space="PSUM") as wop,
            ):
                woT_sb = ap2.tile([128, HL, D], BF16)
                attT = ap2.tile([128, HL, S], BF16)
                nc.sync.dma_start(
                    out=woT_sb[:], in_=woT_t.ap().rearrange("(j p) c -> p j c", p=128))

                # wo matmul groups from the previous chunk, interleaved into
                # the next chunk's s-loops so PE fills exp-wait gaps.
                wo_pending = []
                # deferred normalize/transpose epilogues: drained after the
                # NEXT head's first score pair so their DVE latency hides
                # under PE score work.
                norm_pending = []

                def drain_norms():
                    while norm_pending:
                        aps, hh, tq = norm_pending.pop(0)
                        rinv = stp.tile([128, 1], F32, tag="rinv")
                        nc.vector.reciprocal(rinv[:], aps[:, 128:129])
                        a_sb = esc.tile([128, 128], BF16, tag="asb")
                        nc.vector.tensor_scalar_mul(a_sb[:], aps[:, 0:128], rinv[:])
                        tp = tpp.tile([128, 128], BF16, tag="tp")
                        nc.tensor.transpose(tp[:], a_sb[:], ident[:])
                        nc.vector.tensor_copy(out=attT[:, hh, ts(tq, 128)], in_=tp[:])

                # Each wo group is emitted as two 2-matmul halves so there is
                # enough gap-filler for every exp-wait slot (224 halves vs 140
                # slots; whole groups only number 112).
                wo_cur = []

                def emit_wo(n):
                    while n and (wo_pending or wo_cur):
                        n -= 1
                        if not wo_cur:
                            tq, dc = wo_pending.pop(0)
                            wops = wop.tile([128, 512], F32, tag="wo")
                            for j in (0, 1):
                                nc.tensor.matmul(wops[:], attT[:, j, ts(tq, 128)],
                                                 woT_sb[:, j, ds(dc * 512, 512)],
                                                 start=(j == 0), stop=False)
                            wo_cur.append((tq, dc, wops))
                        else:
                            tq, dc, wops = wo_cur.pop()
                            for j in (2, 3):
                                nc.tensor.matmul(wops[:], attT[:, j, ts(tq, 128)],
                                                 woT_sb[:, j, ds(dc * 512, 512)],
                                                 start=False, stop=(j == HL - 1))
                            y_sb = ysb.tile([128, 512], BF16, tag="y")
                            if dc % 2 == 0:
                                nc.scalar.activation(y_sb[:], wops[:], CPY)
                            else:
                                nc.vector.tensor_copy(out=y_sb[:], in_=wops[:])
                            nc.sync.dma_start(
                                out=y_t[ts(tq, 128), ds(dc * 512, 512)], in_=y_sb[:])

                for c8 in range(8):
                    base_tq = 2 * c8
                    for h in range(HL):
                        nps = c8 + 1  # sk-tile pairs
                        att_ps = [attp.tile([128, 129], F32, tag="attps",
                                            name=f"attps{i}")
                                  for i in range(2)]

                        def emit_sc(sp):
                            # scores^T + exp for sk-tile pair (2sp, 2sp+1).
                            # exp is emitted per block so PV(2sp) only waits
                            # on the first half (subtile deps). On the diagonal
                            # pair, block b=1's ti=0 sub-columns are fully
                            # masked and never read: skip computing them.
                            sc = scp.tile([128, 512], F32, tag="sc")
                            e = esc.tile([128, 512], BF16, tag="e")
                            diag = sp == c8
                            off = 384 if diag else 256
                            nc.tensor.matmul(sc[:, 0:256],
                                             kT[:, h, ts(2 * sp, 128)],
                                             qT[:, h, ds(c8 * 256, 256)],
                                             start=True, stop=True)
                            nc.tensor.matmul(sc[:, off:512],
                                             kT[:, h, ts(2 * sp + 1, 128)],
                                             qT[:, h, ds(c8 * 256 + off - 256, 512 - off)],
                                             start=True, stop=True)
                            if diag:
                                nc.scalar.activation(e[:, 0:256], sc[:, 0:256],
                                                     EXP, scale=SCALE)
                                nc.scalar.activation(e[:, 384:512], sc[:, 384:512],
                                                     EXP, scale=SCALE)
                            else:
                                nc.scalar.activation(e[:], sc[:], EXP, scale=SCALE)
                            if diag:
                                nc.vector.tensor_tensor(out=e[:, 0:128], in0=e[:, 0:128], in1=ltm[:], op=MUL)
                                nc.vector.tensor_tensor(out=e[:, 384:512], in0=e[:, 384:512], in1=ltm[:], op=MUL)
                            return e

                        def emit_pv(sp, e):
                            for b in range(2):
                                s = 2 * sp + b
                                for ti in range(2):
                                    tq = base_tq + ti
                                    if tq >= s:
                                        nc.tensor.matmul(
                                            att_ps[ti][:],
                                            e[:, ds(b * 256 + ti * 128, 128)],
                                            vS[:, s, h, 0:129],
                                            start=(s == 0), stop=(s == tq))

                        # 2-pair-deep score/exp pipeline: PE stays ~2 exp
                        # latencies ahead of the PV consumer.
                        es = [emit_sc(sp) for sp in range(min(2, nps))]
                        drain_norms()
                        for sp in range(nps):
                            if sp + 2 < nps:
                                es.append(emit_sc(sp + 2))
                            emit_pv(sp, es[sp])
                            emit_wo(1)

                        for ti in range(2):
                            norm_pending.append((att_ps[ti], h, base_tq + ti))

                    for ti in range(2):
                        for dc in range(8):
                            wo_pending.append((base_tq + ti, dc))
                drain_norms()
                emit_wo(2 * len(wo_pending) + len(wo_cur))

    nc.compile()
    return nc


def _rope_perm():
    """Row permutation putting even rope components in rows 0-63, odd in 64-127
    of each head block."""
    perm = np.empty(DL, dtype=np.int64)
    for r in range(DL):
        h, p = divmod(r, 128)
        perm[r] = h * 128 + (2 * p if p < 64 else 2 * (p - 64) + 1)
    return perm


def _prep_inputs(x, freqs, wq, wk, wv, wo):
    bf16 = ml_dtypes.bfloat16
    x2 = np.asarray(x, dtype=np.float32).reshape(S, D)
    xT = np.ascontiguousarray(x2.T).astype(bf16)
    f64 = np.asarray(freqs, dtype=np.float64)
    # cosF[p, st*128 + j] = cos(freqs[st*128 + p, j % 64]); pairs duplicated
    # along the free dim so one [128,128] tile covers a whole head.
    cosP = np.cos(f64).reshape(NT, 128, 64)
    sinP = np.sin(f64).reshape(NT, 128, 64)
    cos2 = np.ascontiguousarray(
        np.concatenate([cosP, cosP], axis=2).transpose(1, 0, 2).reshape(128, S)
    ).astype(bf16)
    sin2 = np.ascontiguousarray(
        np.concatenate([sinP, sinP], axis=2).transpose(1, 0, 2).reshape(128, S)
    ).astype(bf16)
    i = np.arange(128)
    ltm = (i[:, None] <= i[None, :]).astype(np.float32).astype(bf16)
    perm = _rope_perm()
    in_maps = []
    for c in range(NCORES):
        sl = slice(c * DL, (c + 1) * DL)
        wq_c = np.asarray(wq, np.float32)[sl, :][perm, :]
        wk_c = np.asarray(wk, np.float32)[sl, :][perm, :]
        wv_c = np.asarray(wv, np.float32)[sl, :]
        wo_c = np.asarray(wo, np.float32)[:, sl]
        in_maps.append({
            "xT": xT,
            "wqT": np.ascontiguousarray(wq_c.T).astype(bf16),
            "wkT": np.ascontiguousarray(wk_c.T).astype(bf16),
            "wvT": np.ascontiguousarray(wv_c.T).astype(bf16),
            "woT": np.ascontiguousarray(wo_c.T).astype(bf16),
            "cosF": cos2,
            "sinF": sin2,
            "ltm": ltm,
        })
    return in_maps


def _run(inputs, trace=False):
    if "nc" not in _CACHE:
        _CACHE["nc"] = _build()
    nc = _CACHE["nc"]
    in_maps = _prep_inputs(**inputs)
    res = run_bass_kernel_spmd(nc, in_maps, core_ids=list(range(NCORES)), trace=trace)
    y = np.zeros((S, D), dtype=np.float64)
    for c in range(NCORES):
        y += res.results[c]["y"].astype(np.float64)
    return y.astype(np.float32).reshape(B, S, D), res.exec_time_ns


def kernel(**inputs):
    y, _ = _run(inputs, trace=False)
    return y
